# revision 6
# baseline (speedup 1.0000x reference)
"""Bahdanau-style additive attention on 8 TRN2 NeuronCores.

  hidden = tanh(q @ Wq + k @ Wk)        (B, L, H)
  scores = hidden @ v_param             (B, L)
  attn   = softmax(scores, axis=-1)
  out    = attn @ v                     (B, D)

Sharding: data-parallel over batch — 4 batches per core (B=32, 8 cores).

All-bf16 pipeline (k, Wk, hidden, w, v in bf16; f32 PSUM accumulate).
Measured end-to-end rel err ~9e-3 vs the fp32 reference (threshold 2e-2).

Two batches are processed as interleaved chunk streams so the Tensor
engine always has an independent chunk to chew while the other stream
waits on its tanh — without this the PE idles a slice of every chunk
and the HAM activity monitor re-throttles its clock to 1.2 GHz.

Per-core device pipeline, per 512-position chunk of each stream:

  W1  pre[H, C]   = Wk.T @ kT           one bf16 matmul
  ACT hh          = tanh(pre + qWq_b)   per-partition bias, bf16 out
  W2  scol[:, j]  = hh_js.T @ vph       4 one-col matmuls; hh stationary
  ACT w = exp(scol4)                    per 4 chunks, [128, 16] -> bf16
  W3  acc[32c, :] = w_col.T @ [v | 1]   4 col-group-packed accumulation
                                        chains (PSUM partitions 0/32/64/96)
                                        right after each exp; the ones col
                                        gives the softmax denominator
  DVE copies acc -> out rows; host sums the 4 partials and divides.
"""

import ml_dtypes
import numpy as np

import concourse.bass as bass
import concourse.mybir as mybir
from concourse.tile import TileContext

B, L, D, H = 32, 8192, 128, 128
NCORES = 8
BPC = B // NCORES  # batches per core
NSTREAM = 2  # interleaved batch streams
SC = 512  # L positions per W1/tanh chunk (1 psum bank)
NSC = L // SC  # 16 chunks per batch
KTILE = 2048  # L positions per kT DMA tile
KSC = KTILE // SC  # chunks per kT tile
SUB = 128  # L positions per W2/W3 sub-chunk (stationary width)
NSUB = SC // SUB  # 4
DV = 130  # v row: 128 data + ones col + 1 pad (4B align)
VT_COLS = 16  # W3 sub-chunks per v SBUF tile
NVT = L // (SUB * VT_COLS)  # 4 v tiles per batch
NCHAIN = 4  # W3 col-group accumulation chains

F32 = mybir.dt.float32
BF16 = mybir.dt.bfloat16
ACTF = mybir.ActivationFunctionType

_CACHE = {}


def _split_excess_waits(nc, max_waits=1):
    """walrus in this env accepts at most one sync-wait per instruction;
    move extras onto InstNoOps placed just before (same engine, in order)."""
    for fn in nc.m.functions:
        for bb in fn.blocks:
            insts = list(bb.instructions)
            new_insts = []
            for ins in insts:
                si = ins.sync_info
                waits = list(si.on_wait) if si and si.on_wait else []
                if len(waits) > max_waits:
                    extra, keep = waits[:-max_waits], waits[-max_waits:]
                    for g0 in range(0, len(extra), max_waits):
                        pre = mybir.InstNoOp(
                            name=f"{ins.name}-waitsplit{g0}",
                            engine=ins.engine,
                            ins=[],
                            outs=[],
                            sync_info=mybir.SyncInfo(
                                on_wait=extra[g0 : g0 + max_waits], on_update=[]
                            ),
                        )
                        nc.register_instruction(pre, overwrite=True)
                        new_insts.append(pre)
                    ins.sync_info = mybir.SyncInfo(
                        on_wait=keep, on_update=list(si.on_update or [])
                    )
                new_insts.append(ins)
            if len(new_insts) != len(insts):
                bb.instructions[:] = new_insts


def build_nc():
    nc = bass.Bass("TRN2")

    kh_in = nc.dram_tensor("kh", [BPC, D, L], BF16, kind="ExternalInput")
    # packed consts: cols 0:4 qwq (f32), 4:68 wk (bf16 pairs), 68 vph|0
    cst_in = nc.dram_tensor("cst", [128, 69], F32, kind="ExternalInput")
    v_in = nc.dram_tensor("vv", [BPC, NVT, SUB, VT_COLS * DV], BF16, kind="ExternalInput")
    out_d = nc.dram_tensor("out", [128, BPC * DV], F32, kind="ExternalOutput")

    with TileContext(nc) as tc:
        with (
            tc.tile_pool(name="const", bufs=1) as cpool,
            tc.tile_pool(name="kp", bufs=8) as kpool,
            tc.tile_pool(name="vp_", bufs=2 * NVT) as vpool,
            tc.tile_pool(name="hp", bufs=4) as hpool,
            tc.tile_pool(name="wp", bufs=4) as wpool,
            tc.tile_pool(name="ob", bufs=1) as opool,
            tc.tile_pool(name="pre", bufs=4, space="PSUM") as pre_pool,
            tc.tile_pool(name="sps", bufs=2, space="PSUM") as s_pool,
            tc.tile_pool(name="ops", bufs=2, space="PSUM") as o_pool,
        ):
            # HAM warm-up on zeroed tiles: needs no DMA, so the PE clock
            # gate lifts during the Tile preamble / first transfers.
            zwarm = cpool.tile([128, 512], BF16)
            nc.gpsimd.memset(zwarm[:], 0.0)
            warm_ps = pre_pool.tile([H, SC], F32, tag="pre")
            for _ in range(16):
                nc.tensor.matmul(
                    warm_ps[:, :512], zwarm[:, :128], zwarm[:], start=True, stop=True
                )

            cst = cpool.tile([128, 69], F32)
            nc.sync.dma_start(cst[:], cst_in[:])
            qwq = cst[:, 0:4]
            wk = cst[:, 4:68].bitcast(BF16)
            vph = cst[:, 68:69].bitcast(BF16)[:, 0:1]

            out_sb = opool.tile([128, BPC * DV], F32)

            def load_ktile(b, g):
                ktile = kpool.tile([D, KTILE], BF16, tag="kt")
                nc.sync.dma_start(ktile[:], kh_in[b, :, g * KTILE : (g + 1) * KTILE])
                return ktile

            def load_vtile(b, g):
                vtile = vpool.tile([SUB, VT_COLS * DV], BF16, tag="vt")
                nc.gpsimd.dma_start(vtile[:], v_in[b, g])
                return vtile

            NPAIR = BPC // NSTREAM
            next_v = None
            for pair in range(NPAIR):
                bs = [pair * NSTREAM + s for s in range(NSTREAM)]
                kts = [{} for _ in range(NSTREAM)]
                if pair == 0:
                    v_tiles = [[None] * NVT for _ in range(NSTREAM)]
                    for s in range(NSTREAM):
                        kts[s][0] = load_ktile(bs[s], 0)
                else:
                    v_tiles = next_v
                next_v = [[None] * NVT for _ in range(NSTREAM)]

                accs, ws, scols = [], [], [None] * NSTREAM
                for s in range(NSTREAM):
                    acc = o_pool.tile([128, DV], F32, tag="acc", name=f"acc{s}")
                    nc.vector.memset(acc[:], 0.0)
                    accs.append(acc)
                    ws.append(wpool.tile([SUB, L // SUB], BF16, tag="w", name=f"w{s}"))

                for t in range(NSC):
                    # v prefetch: this pair's tiles in the front half (one
                    # per stream per 2 chunks), next pair's in the back half
                    if t < 2 * NVT:
                        s, g = t % 2, t // 2
                        if pair == 0:
                            v_tiles[s][g] = load_vtile(bs[s], g)
                    elif pair + 1 < NPAIR:
                        s, g = t % 2, (t - 2 * NVT) // 2
                        next_v[s][g] = load_vtile(bs[s] + NSTREAM, g)

                    for s in range(NSTREAM):
                        b = bs[s]
                        if t % KSC == 0 and t // KSC not in kts[s]:
                            kts[s][t // KSC] = load_ktile(b, t // KSC)

                        pre = pre_pool.tile([H, SC], F32, tag="pre")
                        kt = kts[s][t // KSC]
                        off = (t % KSC) * SC
                        nc.tensor.matmul(
                            pre[:], wk[:], kt[:, off : off + SC],
                            start=True, stop=True,
                        )

                        hh = hpool.tile([H, SC], BF16, tag="hh")
                        nc.scalar.activation(
                            hh[:], pre[:], ACTF.Tanh,
                            bias=qwq[:, b : b + 1], scale=1.0,
                        )
                        if t % 4 == 0:
                            scols[s] = s_pool.tile(
                                [SUB, 4 * NSUB], F32, tag="scol", name=f"scol{s}"
                            )
                        scol = scols[s][:, (t % 4) * NSUB : (t % 4 + 1) * NSUB]
                        for j in range(NSUB):
                            js = slice(j * SUB, (j + 1) * SUB)
                            nc.tensor.matmul(
                                scol[:, j : j + 1], hh[:, js], vph[:],
                                start=True, stop=True,
                            )
                        if t % 4 == 3:
                            g = t // 4  # w col group == v tile index
                            nc.scalar.activation(
                                ws[s][:, 16 * g : 16 * (g + 1)], scols[s][:],
                                ACTF.Exp,
                            )
                            for col in range(VT_COLS):
                                tp = 16 * g + col
                                c = tp % NCHAIN
                                nc.tensor.matmul(
                                    accs[s][32 * c : 32 * c + 1, :],
                                    ws[s][:, tp : tp + 1],
                                    v_tiles[s][g][:, col * DV : (col + 1) * DV],
                                    start=(tp < NCHAIN),
                                    stop=(tp >= L // SUB - NCHAIN),
                                    tile_position=(0, 32 * c),
                                )
                for s in range(NSTREAM):
                    nc.vector.tensor_copy(
                        out_sb[:, bs[s] * DV : (bs[s] + 1) * DV], accs[s][:]
                    )

            nc.sync.dma_start(out_d[:], out_sb[:])

    _split_excess_waits(nc)
    return nc


def _prep_inputs(q, k, v, W_line, v_param):
    """Host-side shard + layout prep. Returns per-core input maps."""
    bf = ml_dtypes.bfloat16
    qWq = q.astype(np.float64) @ W_line[:D].astype(np.float64)  # (B, H)
    wk = np.ascontiguousarray(W_line[D:]).astype(bf)  # (D, H) bf16
    vp_pad = np.zeros((H, 2), dtype=bf)
    vp_pad[:, 0] = v_param.astype(bf)

    cst_base = np.zeros((128, 69), dtype=np.float32)
    cst_base[:, 4:68] = wk.view(np.float32)
    cst_base[:, 68:69] = vp_pad.view(np.float32)

    in_maps = []
    for c in range(NCORES):
        bs = slice(c * BPC, (c + 1) * BPC)
        kh = np.ascontiguousarray(k[bs].transpose(0, 2, 1).astype(bf))  # (BPC, D, L)
        vv = np.zeros((BPC, L, DV), dtype=np.float32)
        vv[:, :, :D] = v[bs]
        vv[:, :, D] = 1.0
        # permute into the SBUF tile layout: [b][vt][p][t*DV+d]
        vv = np.ascontiguousarray(
            vv.reshape(BPC, NVT, VT_COLS, SUB, DV)
            .transpose(0, 1, 3, 2, 4)
            .reshape(BPC, NVT, SUB, VT_COLS * DV)
            .astype(bf)
        )
        cst = cst_base.copy()
        cst[:, 0:4] = qWq[bs].T.astype(np.float32)  # (H, BPC)
        in_maps.append({"kh": kh, "vv": vv, "cst": cst})
    return in_maps


def _gather_output(results):
    out = np.empty((B, D), dtype=np.float32)
    for c, r in enumerate(results):
        rows = r["out"].reshape(128, BPC, DV).astype(np.float64)
        # sum the 4 col-group partial accumulators (partitions 0/32/64/96)
        s = rows[0] + rows[32] + rows[64] + rows[96]  # (BPC, DV)
        out[c * BPC : (c + 1) * BPC] = (s[:, :D] / s[:, D : D + 1]).astype(np.float32)
    return out


def run(q, k, v, W_line, v_param, trace=False, **spmd_kwargs):
    from concourse.bass_utils import run_bass_kernel_spmd

    if "nc" not in _CACHE:
        _CACHE["nc"] = build_nc()
    nc = _CACHE["nc"]
    in_maps = _prep_inputs(q, k, v, W_line, v_param)
    res = run_bass_kernel_spmd(
        nc, in_maps, list(range(NCORES)), trace=trace, **spmd_kwargs
    )
    return _gather_output(res.results), res


def kernel(q, k, v, W_line, v_param):
    out, _ = run(q, k, v, W_line, v_param, trace=False)
    return out
